# revision 19
# baseline (speedup 1.0000x reference)
"""Trainium2 Bass kernel for nn_LocallyDense (grouped gather + per-group Dense
+ LeakyReLU + BatchNorm inference).

Sharding: expert-parallel over groups 0-39 (5 per core on 8 cores), plus a
K-split of group 40: each core computes a 192-row slice of its 1536-long
contraction (padded to 2 K-tiles with zeros) and stores the raw bf16
partial; the host sums the 8 partials and applies bias/LeakyReLU/BN for
that group. Every core runs the identical program shape.

The gather (x columns per group) and all BN constant math happen on the
HOST during input prep — the device program is a pure streamed GEMM in the
transposed formulation out^T[o, b] = W^T x^T:
  - lhsT (stationary) = W K-tile  [K=128, M=128 output-half]
  - rhs  (moving)     = gathered-x K-tile [K=128, N=256 batch]
  - PSUM accumulates the group's K-tiles per output-half
Gathered-x and W K-tiles are interleaved host-side into one combined DRAM
tensor, loaded in 13 chunks on the Sync HWDGE queue in the same order the
PE consumes them. The chunk plan is front/back tapered: a small first
chunk (the group-40 mini slice) plus 4-ktile g0 chunks let compute start
early; the final group tapers 6/4/2 ktiles so almost no work remains after
the last load packet arrives. A short burst of warmup matmuls on a memset
tile keeps the PE clock up between the entry barrier and the first real
chunk (avoids the idle -> pstate-downshift -> slow-matmul ramp).

Epilogue per group: ACT Prelu (bias as per-partition scalar AP — the
transposed layout puts output features on partitions) then one DVE
tensor_scalar for the BN affine, with inv = gamma/sqrt(var+eps),
c = beta - mean*inv precomputed on host. Both output halves collect into
one [128, 2, B] SBUF tile stored with a single DMA per group (DRAM layout
is partition-major to match). The last group's halves store separately on
the then-idle Sync queue so the first half's store overlaps the second
half's epilogue. Outputs are bf16, cast/transposed back on the host.
"""

import numpy as np
import ml_dtypes

B, D_IN, N_GROUPS, G, D_OUT = 256, 65536, 41, 1536, 256
BN_EPS = 1e-3
ALPHA = 0.3
N_CORES = 8
NGF = 5               # full groups per core
KT = G // 128         # 12 K-tiles per full group
MKT = 2               # K-tiles in the group-40 mini slice (192 rows + pad)
MROWS = G // N_CORES  # 192 real contraction rows per core for group 40
CW = B + D_OUT        # combined tile width per K-tile (x cols + w cols)

# (group, n_ktiles) in load order; -1 = group-40 mini slice
CHUNK_PLAN = [(-1, 2),
              (0, 4), (0, 4), (0, 4),
              (1, 6), (1, 6),
              (2, 6), (2, 6),
              (3, 6), (3, 6),
              (4, 6), (4, 4), (4, 2)]
TOTAL_KT = sum(kt for _, kt in CHUNK_PLAN)
assert TOTAL_KT == MKT + NGF * KT

WARMUP = 12           # PE warmup matmuls before first real chunk

TRACE = False
TRACE_KW = {}
REPEAT = 1

_prog_cache = {}


def _build_program(fused):
    """fused=True: BN folds into the ACT (valid when c == 0 and inv >= 0:
    inv*Prelu(ps+b)+c == Prelu(inv*ps + inv*b)); single ACT per half,
    no tensor_scalar. fused=False: generic ACT + TS affine."""
    import concourse.bacc as bacc
    import concourse.mybir as mybir
    import concourse.tile as tile

    f32 = mybir.dt.float32
    dt_x = mybir.dt.bfloat16

    nc = bacc.Bacc("TRN2", target_bir_lowering=False, debug=False,
                   num_devices=N_CORES)
    xw = nc.dram_tensor("xw", [128, TOTAL_KT * CW], dt_x,
                        kind="ExternalInput")
    # cols 0-9: bias[g, h*128+p] (g<5); 12-13: inv[h*128+p]; 14-15: c
    cons = nc.dram_tensor("cons", [128, 16], f32, kind="ExternalInput")
    out = nc.dram_tensor("out", [128, NGF * 2 * B], dt_x,
                         kind="ExternalOutput")
    out40 = nc.dram_tensor("out40", [128, 2 * B], dt_x,
                           kind="ExternalOutput")

    offs = []
    o = 0
    for _, kt in CHUNK_PLAN:
        offs.append(o)
        o += kt

    with tile.TileContext(nc) as tc:
        with tc.tile_pool(name="xw", bufs=1) as xwpool, \
             tc.tile_pool(name="ep", bufs=2) as epool, \
             tc.tile_pool(name="ps", bufs=3, space="PSUM") as ppool, \
             tc.tile_pool(name="wps", bufs=1, space="PSUM") as wpool:

            ct = xwpool.tile([128, 16], f32, tag="ct")
            nc.scalar.dma_start(out=ct[:], in_=cons[:, :])

            if WARMUP:
                wu = xwpool.tile([128, 256], dt_x, tag="wu")
                nc.gpsimd.memset(wu[:], 1.0)
                wps = wpool.tile([128, 256], f32, tag="wps")
                for i in range(WARMUP):
                    nc.tensor.matmul(out=wps[:], lhsT=wu[:, 0:128],
                                     rhs=wu[:], start=True, stop=True)

            for rep in range(REPEAT):
                # all load triggers up front, one queue, in PE order
                chunks = []
                for ci, (g, kt) in enumerate(CHUNK_PLAN):
                    xwt = xwpool.tile([128, kt, CW], dt_x, tag=f"ck{ci}",
                                      name=f"ck{ci}_{rep}")
                    base = offs[ci] * CW
                    if g == -1:
                        # group-40 slice is 192 rows: k-tile 0 is full,
                        # k-tile 1 holds only 64 valid rows — skip
                        # transferring the 64 zero-padding partitions
                        nc.sync.dma_start(out=xwt[:, 0, :],
                                          in_=xw[:, base:base + CW])
                        nc.sync.dma_start(
                            out=xwt[0:64, 1, :],
                            in_=xw[0:64, base + CW:base + 2 * CW])
                    else:
                        nc.sync.dma_start(out=xwt[:],
                                          in_=xw[:, base:base + kt * CW])
                    chunks.append(xwt)

                cur_ps = {}
                kdone = {}
                for ci, (g, kt) in enumerate(CHUNK_PLAN):
                    xwt = chunks[ci]
                    nkt = MKT if g == -1 else KT
                    if g not in cur_ps:
                        cur_ps[g] = [ppool.tile([128, B], f32, tag=f"ps{h}",
                                                name=f"ps{h}_{rep}_{g}")
                                     for h in range(2)]
                        kdone[g] = 0
                    pss = cur_ps[g]
                    k0 = kdone[g]
                    if ci == len(CHUNK_PLAN) - 1:
                        # final chunk: h-major, h1 first, so pss[1] stops
                        # 2 MMs early and its ACT/store overlap the h0 MMs
                        for h in (1, 0):
                            for c in range(kt):
                                nc.tensor.matmul(
                                    out=pss[h][:],
                                    lhsT=xwt[:, c,
                                             B + h * 128:B + (h + 1) * 128],
                                    rhs=xwt[:, c, 0:B],
                                    start=(k0 + c == 0),
                                    stop=(k0 + c == nkt - 1))
                    else:
                        for c in range(kt):
                            # mini k-tile 1 carries only 64 valid rows
                            r = 64 if (g == -1 and c == 1) else 128
                            for h in range(2):
                                nc.tensor.matmul(
                                    out=pss[h][:],
                                    lhsT=xwt[0:r, c,
                                             B + h * 128:B + (h + 1) * 128],
                                    rhs=xwt[0:r, c, 0:B],
                                    start=(k0 + c == 0),
                                    stop=(k0 + c == nkt - 1))
                    kdone[g] += kt
                    if kdone[g] < nkt:
                        continue
                    # group complete -> epilogue + merged store
                    if g == -1:
                        ym = epool.tile([128, 2, B], dt_x, tag="ym",
                                        name=f"ym_{rep}")
                        for h in range(2):
                            nc.vector.tensor_copy(ym[:, h, :], pss[h][:])
                        nc.scalar.dma_start(out=out40[:, :], in_=ym[:])
                    else:
                        y2 = epool.tile([128, 2, B], dt_x, tag="y2",
                                        name=f"y2_{rep}_{g}")
                        last_g = (g == NGF - 1)
                        for h in ((1, 0) if last_g else (0, 1)):
                            if fused and last_g and h == 1:
                                # h1's psum stops 2 MMs early (h-major
                                # order): give it the slower DVE chain,
                                # concurrent with h0's ACT on scalar:
                                # u = inv*ps + inv*b, LeakyReLU = max(a*u,u)
                                u = epool.tile([128, B], dt_x, tag="t",
                                               name=f"u_{rep}_{g}")
                                nc.vector.tensor_scalar(
                                    out=u[:], in0=pss[h][:],
                                    scalar1=ct[:, 12 + h:13 + h],
                                    scalar2=ct[:, 2 * g + h:2 * g + h + 1],
                                    op0=mybir.AluOpType.mult,
                                    op1=mybir.AluOpType.add)
                                nc.vector.scalar_tensor_tensor(
                                    out=y2[:, h, :], in0=u[:],
                                    scalar=float(ALPHA), in1=u[:],
                                    op0=mybir.AluOpType.mult,
                                    op1=mybir.AluOpType.max)
                            elif fused:
                                # cons cols 0-9 hold inv*b; 12-13 hold inv
                                nc.scalar.activation(
                                    out=y2[:, h, :], in_=pss[h][:],
                                    func=mybir.ActivationFunctionType.Prelu,
                                    bias=ct[:, 2 * g + h:2 * g + h + 1],
                                    scale=ct[:, 12 + h:13 + h],
                                    alpha=float(ALPHA))
                            else:
                                # bf16 intermediate: 2x DVE rate on the BN
                                # affine; extra rounding ~0.2% (gate 2e-2)
                                t = epool.tile([128, B], dt_x, tag="t",
                                               name=f"t_{rep}_{g}_{h}")
                                nc.scalar.activation(
                                    out=t[:], in_=pss[h][:],
                                    func=mybir.ActivationFunctionType.Prelu,
                                    bias=ct[:, 2 * g + h:2 * g + h + 1],
                                    scale=1.0, alpha=float(ALPHA))
                                nc.vector.tensor_scalar(
                                    out=y2[:, h, :], in0=t[:],
                                    scalar1=ct[:, 12 + h:13 + h],
                                    scalar2=ct[:, 14 + h:15 + h],
                                    op0=mybir.AluOpType.mult,
                                    op1=mybir.AluOpType.add)
                            if last_g:
                                # tail: per-half stores on separate queues
                                # (h1 -> idle sync, h0 -> scalar right
                                # after its ACT) so the triggers and
                                # flights overlap
                                eng = nc.sync if h == 1 else nc.scalar
                                eng.dma_start(
                                    out=out[:, (g * 2 + h) * B:
                                            (g * 2 + h + 1) * B],
                                    in_=y2[:, h, :])
                        if not last_g:
                            nc.scalar.dma_start(
                                out=out[:, g * 2 * B:(g + 1) * 2 * B],
                                in_=y2[:])
    nc.compile()
    return nc


def _get_program(fused):
    key = (WARMUP, REPEAT, tuple(CHUNK_PLAN), fused)
    if key not in _prog_cache:
        _prog_cache[key] = _build_program(fused)
    return _prog_cache[key]


def _prep_inputs(x, gidx, W, b, gamma, beta, mmean, mvar, fused):
    dtx = ml_dtypes.bfloat16
    inv = (gamma.astype(np.float64) /
           np.sqrt(mvar.astype(np.float64) + BN_EPS)).astype(np.float32)
    cvec = (beta - mmean * inv).astype(np.float32)
    beff = b * inv[None, :] if fused else b   # ACT bias per mode
    inv_pc = inv.reshape(2, 128).T      # [128, 2]
    c_pc = cvec.reshape(2, 128).T       # [128, 2]
    A40 = x[:, gidx[40]]                # [B, G] group-40 gather
    W40 = W[40]                         # [G, D_OUT]

    offs = []
    o = 0
    for _, kt in CHUNK_PLAN:
        offs.append(o)
        o += kt

    in_maps, metas = [], []
    for cid in range(N_CORES):
        gs = list(range(5 * cid, 5 * cid + 5))
        xwb = np.zeros((128, TOTAL_KT, CW), dtype=dtx)
        for gi, grp in enumerate(gs):
            A = x[:, gidx[grp]]            # [B, G]
            At = A.T.reshape(KT, 128, B)   # [kt, p, b]
            Wt = W[grp].reshape(KT, 128, D_OUT)
            kpos = 0
            for ci, (g, kt) in enumerate(CHUNK_PLAN):
                if g != gi:
                    continue
                for j in range(kt):
                    xwb[:, offs[ci] + j, :B] = At[kpos + j]
                    xwb[:, offs[ci] + j, B:] = Wt[kpos + j]
                kpos += kt
        # mini: group-40 K-slice rows [MROWS*cid, MROWS*(cid+1)) pad to MKT
        sl = slice(MROWS * cid, MROWS * (cid + 1))
        mx = np.zeros((MKT * 128, B), np.float32)
        mw = np.zeros((MKT * 128, D_OUT), np.float32)
        mx[:MROWS] = A40.T[sl]
        mw[:MROWS] = W40[sl]
        mini_ci = next(ci for ci, (g, _) in enumerate(CHUNK_PLAN) if g == -1)
        for j in range(MKT):
            xwb[:, offs[mini_ci] + j, :B] = mx[j * 128:(j + 1) * 128]
            xwb[:, offs[mini_ci] + j, B:] = mw[j * 128:(j + 1) * 128]

        consb = np.zeros((128, 16), np.float32)
        consb[:, 0:10] = beff[gs].reshape(NGF, 2, 128).transpose(2, 0, 1) \
                                 .reshape(128, 10)
        consb[:, 12:14] = inv_pc
        consb[:, 14:16] = c_pc
        in_maps.append({"xw": xwb.reshape(128, TOTAL_KT * CW),
                        "cons": np.ascontiguousarray(consb)})
        metas.append(gs)
    return in_maps, metas


def _unshard(results, metas, b, gamma, beta, mmean, mvar):
    full = np.empty((B, N_GROUPS, D_OUT), dtype=np.float32)
    z40 = np.zeros((D_OUT, B), np.float32)
    for cid, gs in enumerate(metas):
        o = results[cid]["out"].astype(np.float32) \
            .reshape(128, NGF, 2, B)
        full[:, gs, :] = o.transpose(3, 1, 2, 0).reshape(B, NGF, D_OUT)
        o40 = results[cid]["out40"].astype(np.float32).reshape(128, 2, B)
        z40 += o40.transpose(1, 0, 2).reshape(D_OUT, B)
    inv = (gamma.astype(np.float64) /
           np.sqrt(mvar.astype(np.float64) + BN_EPS)).astype(np.float32)
    cvec = (beta - mmean * inv).astype(np.float32)
    z = z40 + b[40][:, None]
    t = np.where(z >= 0, z, ALPHA * z)
    full[:, 40, :] = (t * inv[:, None] + cvec[:, None]).T
    return full


def kernel(**inputs):
    x = np.asarray(inputs["x"], dtype=np.float32)
    gidx = np.asarray(inputs["group_idx"]).astype(np.int64)
    W = np.asarray(inputs["W"], dtype=np.float32)
    b = np.asarray(inputs["b"], dtype=np.float32)
    gamma = np.asarray(inputs["gamma"], dtype=np.float32)
    beta = np.asarray(inputs["beta"], dtype=np.float32)
    mmean = np.asarray(inputs["moving_mean"], dtype=np.float32)
    mvar = np.asarray(inputs["moving_var"], dtype=np.float32)

    inv = (gamma.astype(np.float64) /
           np.sqrt(mvar.astype(np.float64) + BN_EPS)).astype(np.float32)
    cvec = (beta - mmean * inv).astype(np.float32)
    # BN folds into the ACT exactly when the affine addend is zero and the
    # scale is non-negative (Prelu commutes with positive scaling)
    fused = bool(np.all(cvec == 0.0) and np.all(inv >= 0.0))

    in_maps, metas = _prep_inputs(x, gidx, W, b, gamma, beta, mmean, mvar,
                                  fused)
    nc = _get_program(fused)

    from concourse import bass_utils
    res = bass_utils.run_bass_kernel_spmd(
        nc, in_maps, core_ids=list(range(N_CORES)), trace=TRACE, **TRACE_KW)
    if TRACE:
        kernel.last_result = res

    return _unshard(res.results, metas, b, gamma, beta, mmean, mvar)


# revision 20
# speedup vs baseline: 1.0117x; 1.0117x over previous
"""Trainium2 Bass kernel for nn_LocallyDense (grouped gather + per-group Dense
+ LeakyReLU + BatchNorm inference).

Sharding: expert-parallel over groups 0-39 (5 per core on 8 cores), plus a
K-split of group 40: each core computes a 192-row slice of its 1536-long
contraction (padded to 2 K-tiles with zeros) and stores the raw bf16
partial; the host sums the 8 partials and applies bias/LeakyReLU/BN for
that group. Every core runs the identical program shape.

The gather (x columns per group) and all BN constant math happen on the
HOST during input prep — the device program is a pure streamed GEMM in the
transposed formulation out^T[o, b] = W^T x^T:
  - lhsT (stationary) = W K-tile  [K=128, M=128 output-half]
  - rhs  (moving)     = gathered-x K-tile [K=128, N=256 batch]
  - PSUM accumulates the group's K-tiles per output-half
Gathered-x and W K-tiles are interleaved host-side into one combined DRAM
tensor, loaded in 13 chunks on the Sync HWDGE queue in the same order the
PE consumes them. The chunk plan is front/back tapered: a small first
chunk (the group-40 mini slice) plus 4-ktile g0 chunks let compute start
early; the final group tapers 6/4/2 ktiles so almost no work remains after
the last load packet arrives. A short burst of warmup matmuls on a memset
tile keeps the PE clock up between the entry barrier and the first real
chunk (avoids the idle -> pstate-downshift -> slow-matmul ramp).

Epilogue per group: ACT Prelu (bias as per-partition scalar AP — the
transposed layout puts output features on partitions) then one DVE
tensor_scalar for the BN affine, with inv = gamma/sqrt(var+eps),
c = beta - mean*inv precomputed on host. Both output halves collect into
one [128, 2, B] SBUF tile stored with a single DMA per group (DRAM layout
is partition-major to match). The last group's halves store separately on
the then-idle Sync queue so the first half's store overlaps the second
half's epilogue. Outputs are bf16, cast/transposed back on the host.
"""

import numpy as np
import ml_dtypes

B, D_IN, N_GROUPS, G, D_OUT = 256, 65536, 41, 1536, 256
BN_EPS = 1e-3
ALPHA = 0.3
N_CORES = 8
NGF = 5               # full groups per core
KT = G // 128         # 12 K-tiles per full group
MKT = 2               # K-tiles in the group-40 mini slice (192 rows + pad)
MROWS = G // N_CORES  # 192 real contraction rows per core for group 40
CW = B + D_OUT        # combined tile width per K-tile (x cols + w cols)

# (group, n_ktiles) in load order; -1 = group-40 mini slice
CHUNK_PLAN = [(-1, 2),
              (0, 4), (0, 4), (0, 4),
              (1, 6), (1, 6),
              (2, 6), (2, 6),
              (3, 6), (3, 6),
              (4, 6), (4, 4), (4, 2)]
TOTAL_KT = sum(kt for _, kt in CHUNK_PLAN)
assert TOTAL_KT == MKT + NGF * KT

WARMUP = 6            # PE warmup matmuls before first real chunk

TRACE = False
TRACE_KW = {}
REPEAT = 1

_prog_cache = {}


def _build_program(fused):
    """fused=True: BN folds into the ACT (valid when c == 0 and inv >= 0:
    inv*Prelu(ps+b)+c == Prelu(inv*ps + inv*b)); single ACT per half,
    no tensor_scalar. fused=False: generic ACT + TS affine."""
    import concourse.bacc as bacc
    import concourse.mybir as mybir
    import concourse.tile as tile

    f32 = mybir.dt.float32
    dt_x = mybir.dt.bfloat16

    nc = bacc.Bacc("TRN2", target_bir_lowering=False, debug=False,
                   num_devices=N_CORES)
    xw = nc.dram_tensor("xw", [128, TOTAL_KT * CW], dt_x,
                        kind="ExternalInput")
    # cols 0-9: bias[g, h*128+p] (g<5); 12-13: inv[h*128+p]; 14-15: c
    cons = nc.dram_tensor("cons", [128, 16], f32, kind="ExternalInput")
    out = nc.dram_tensor("out", [128, NGF * 2 * B], dt_x,
                         kind="ExternalOutput")
    out40 = nc.dram_tensor("out40", [128, 2 * B], dt_x,
                           kind="ExternalOutput")

    offs = []
    o = 0
    for _, kt in CHUNK_PLAN:
        offs.append(o)
        o += kt

    with tile.TileContext(nc) as tc:
        with tc.tile_pool(name="xw", bufs=1) as xwpool, \
             tc.tile_pool(name="ep", bufs=2) as epool, \
             tc.tile_pool(name="ps", bufs=3, space="PSUM") as ppool, \
             tc.tile_pool(name="wps", bufs=1, space="PSUM") as wpool:

            ct = xwpool.tile([128, 16], f32, tag="ct")
            nc.scalar.dma_start(out=ct[:], in_=cons[:, :])

            if WARMUP:
                wu = xwpool.tile([128, 256], dt_x, tag="wu")
                nc.gpsimd.memset(wu[:], 1.0)
                wps = wpool.tile([128, 256], f32, tag="wps")
                for i in range(WARMUP):
                    nc.tensor.matmul(out=wps[:], lhsT=wu[:, 0:128],
                                     rhs=wu[:], start=True, stop=True)

            for rep in range(REPEAT):
                # all load triggers up front, one queue, in PE order
                chunks = []
                for ci, (g, kt) in enumerate(CHUNK_PLAN):
                    xwt = xwpool.tile([128, kt, CW], dt_x, tag=f"ck{ci}",
                                      name=f"ck{ci}_{rep}")
                    base = offs[ci] * CW
                    if g == -1:
                        # group-40 slice is 192 rows: k-tile 0 is full,
                        # k-tile 1 holds only 64 valid rows — skip
                        # transferring the 64 zero-padding partitions
                        nc.sync.dma_start(out=xwt[:, 0, :],
                                          in_=xw[:, base:base + CW])
                        nc.sync.dma_start(
                            out=xwt[0:64, 1, :],
                            in_=xw[0:64, base + CW:base + 2 * CW])
                    else:
                        nc.sync.dma_start(out=xwt[:],
                                          in_=xw[:, base:base + kt * CW])
                    chunks.append(xwt)

                cur_ps = {}
                kdone = {}
                for ci, (g, kt) in enumerate(CHUNK_PLAN):
                    xwt = chunks[ci]
                    nkt = MKT if g == -1 else KT
                    if g not in cur_ps:
                        cur_ps[g] = [ppool.tile([128, B], f32, tag=f"ps{h}",
                                                name=f"ps{h}_{rep}_{g}")
                                     for h in range(2)]
                        kdone[g] = 0
                    pss = cur_ps[g]
                    k0 = kdone[g]
                    if ci == len(CHUNK_PLAN) - 1:
                        # final chunk: h-major, h1 first, so pss[1] stops
                        # 2 MMs early and its ACT/store overlap the h0 MMs
                        for h in (1, 0):
                            for c in range(kt):
                                nc.tensor.matmul(
                                    out=pss[h][:],
                                    lhsT=xwt[:, c,
                                             B + h * 128:B + (h + 1) * 128],
                                    rhs=xwt[:, c, 0:B],
                                    start=(k0 + c == 0),
                                    stop=(k0 + c == nkt - 1))
                    else:
                        for c in range(kt):
                            # mini k-tile 1 carries only 64 valid rows
                            r = 64 if (g == -1 and c == 1) else 128
                            for h in range(2):
                                nc.tensor.matmul(
                                    out=pss[h][:],
                                    lhsT=xwt[0:r, c,
                                             B + h * 128:B + (h + 1) * 128],
                                    rhs=xwt[0:r, c, 0:B],
                                    start=(k0 + c == 0),
                                    stop=(k0 + c == nkt - 1))
                    kdone[g] += kt
                    if kdone[g] < nkt:
                        continue
                    # group complete -> epilogue + merged store
                    if g == -1:
                        ym = epool.tile([128, 2, B], dt_x, tag="ym",
                                        name=f"ym_{rep}")
                        for h in range(2):
                            nc.vector.tensor_copy(ym[:, h, :], pss[h][:])
                        nc.scalar.dma_start(out=out40[:, :], in_=ym[:])
                    else:
                        y2 = epool.tile([128, 2, B], dt_x, tag="y2",
                                        name=f"y2_{rep}_{g}")
                        last_g = (g == NGF - 1)
                        for h in ((1, 0) if last_g else (0, 1)):
                            if fused and last_g and h == 1:
                                # h1's psum stops 2 MMs early (h-major
                                # order): give it the slower DVE chain,
                                # concurrent with h0's ACT on scalar:
                                # u = inv*ps + inv*b, LeakyReLU = max(a*u,u)
                                u = epool.tile([128, B], dt_x, tag="t",
                                               name=f"u_{rep}_{g}")
                                nc.vector.tensor_scalar(
                                    out=u[:], in0=pss[h][:],
                                    scalar1=ct[:, 12 + h:13 + h],
                                    scalar2=ct[:, 2 * g + h:2 * g + h + 1],
                                    op0=mybir.AluOpType.mult,
                                    op1=mybir.AluOpType.add)
                                nc.vector.scalar_tensor_tensor(
                                    out=y2[:, h, :], in0=u[:],
                                    scalar=float(ALPHA), in1=u[:],
                                    op0=mybir.AluOpType.mult,
                                    op1=mybir.AluOpType.max)
                            elif fused:
                                # cons cols 0-9 hold inv*b; 12-13 hold inv
                                nc.scalar.activation(
                                    out=y2[:, h, :], in_=pss[h][:],
                                    func=mybir.ActivationFunctionType.Prelu,
                                    bias=ct[:, 2 * g + h:2 * g + h + 1],
                                    scale=ct[:, 12 + h:13 + h],
                                    alpha=float(ALPHA))
                            else:
                                # bf16 intermediate: 2x DVE rate on the BN
                                # affine; extra rounding ~0.2% (gate 2e-2)
                                t = epool.tile([128, B], dt_x, tag="t",
                                               name=f"t_{rep}_{g}_{h}")
                                nc.scalar.activation(
                                    out=t[:], in_=pss[h][:],
                                    func=mybir.ActivationFunctionType.Prelu,
                                    bias=ct[:, 2 * g + h:2 * g + h + 1],
                                    scale=1.0, alpha=float(ALPHA))
                                nc.vector.tensor_scalar(
                                    out=y2[:, h, :], in0=t[:],
                                    scalar1=ct[:, 12 + h:13 + h],
                                    scalar2=ct[:, 14 + h:15 + h],
                                    op0=mybir.AluOpType.mult,
                                    op1=mybir.AluOpType.add)
                            if last_g:
                                # tail: per-half stores on separate queues
                                # (h1 -> idle sync, h0 -> scalar right
                                # after its ACT) so the triggers and
                                # flights overlap
                                eng = nc.sync if h == 1 else nc.scalar
                                eng.dma_start(
                                    out=out[:, (g * 2 + h) * B:
                                            (g * 2 + h + 1) * B],
                                    in_=y2[:, h, :])
                        if not last_g:
                            nc.scalar.dma_start(
                                out=out[:, g * 2 * B:(g + 1) * 2 * B],
                                in_=y2[:])
    nc.compile()
    return nc


def _get_program(fused):
    key = (WARMUP, REPEAT, tuple(CHUNK_PLAN), fused)
    if key not in _prog_cache:
        _prog_cache[key] = _build_program(fused)
    return _prog_cache[key]


def _prep_inputs(x, gidx, W, b, gamma, beta, mmean, mvar, fused):
    dtx = ml_dtypes.bfloat16
    inv = (gamma.astype(np.float64) /
           np.sqrt(mvar.astype(np.float64) + BN_EPS)).astype(np.float32)
    cvec = (beta - mmean * inv).astype(np.float32)
    beff = b * inv[None, :] if fused else b   # ACT bias per mode
    inv_pc = inv.reshape(2, 128).T      # [128, 2]
    c_pc = cvec.reshape(2, 128).T       # [128, 2]
    A40 = x[:, gidx[40]]                # [B, G] group-40 gather
    W40 = W[40]                         # [G, D_OUT]

    offs = []
    o = 0
    for _, kt in CHUNK_PLAN:
        offs.append(o)
        o += kt

    in_maps, metas = [], []
    for cid in range(N_CORES):
        gs = list(range(5 * cid, 5 * cid + 5))
        xwb = np.zeros((128, TOTAL_KT, CW), dtype=dtx)
        for gi, grp in enumerate(gs):
            A = x[:, gidx[grp]]            # [B, G]
            At = A.T.reshape(KT, 128, B)   # [kt, p, b]
            Wt = W[grp].reshape(KT, 128, D_OUT)
            kpos = 0
            for ci, (g, kt) in enumerate(CHUNK_PLAN):
                if g != gi:
                    continue
                for j in range(kt):
                    xwb[:, offs[ci] + j, :B] = At[kpos + j]
                    xwb[:, offs[ci] + j, B:] = Wt[kpos + j]
                kpos += kt
        # mini: group-40 K-slice rows [MROWS*cid, MROWS*(cid+1)) pad to MKT
        sl = slice(MROWS * cid, MROWS * (cid + 1))
        mx = np.zeros((MKT * 128, B), np.float32)
        mw = np.zeros((MKT * 128, D_OUT), np.float32)
        mx[:MROWS] = A40.T[sl]
        mw[:MROWS] = W40[sl]
        mini_ci = next(ci for ci, (g, _) in enumerate(CHUNK_PLAN) if g == -1)
        for j in range(MKT):
            xwb[:, offs[mini_ci] + j, :B] = mx[j * 128:(j + 1) * 128]
            xwb[:, offs[mini_ci] + j, B:] = mw[j * 128:(j + 1) * 128]

        consb = np.zeros((128, 16), np.float32)
        consb[:, 0:10] = beff[gs].reshape(NGF, 2, 128).transpose(2, 0, 1) \
                                 .reshape(128, 10)
        consb[:, 12:14] = inv_pc
        consb[:, 14:16] = c_pc
        in_maps.append({"xw": xwb.reshape(128, TOTAL_KT * CW),
                        "cons": np.ascontiguousarray(consb)})
        metas.append(gs)
    return in_maps, metas


def _unshard(results, metas, b, gamma, beta, mmean, mvar):
    full = np.empty((B, N_GROUPS, D_OUT), dtype=np.float32)
    z40 = np.zeros((D_OUT, B), np.float32)
    for cid, gs in enumerate(metas):
        o = results[cid]["out"].astype(np.float32) \
            .reshape(128, NGF, 2, B)
        full[:, gs, :] = o.transpose(3, 1, 2, 0).reshape(B, NGF, D_OUT)
        o40 = results[cid]["out40"].astype(np.float32).reshape(128, 2, B)
        z40 += o40.transpose(1, 0, 2).reshape(D_OUT, B)
    inv = (gamma.astype(np.float64) /
           np.sqrt(mvar.astype(np.float64) + BN_EPS)).astype(np.float32)
    cvec = (beta - mmean * inv).astype(np.float32)
    z = z40 + b[40][:, None]
    t = np.where(z >= 0, z, ALPHA * z)
    full[:, 40, :] = (t * inv[:, None] + cvec[:, None]).T
    return full


def kernel(**inputs):
    x = np.asarray(inputs["x"], dtype=np.float32)
    gidx = np.asarray(inputs["group_idx"]).astype(np.int64)
    W = np.asarray(inputs["W"], dtype=np.float32)
    b = np.asarray(inputs["b"], dtype=np.float32)
    gamma = np.asarray(inputs["gamma"], dtype=np.float32)
    beta = np.asarray(inputs["beta"], dtype=np.float32)
    mmean = np.asarray(inputs["moving_mean"], dtype=np.float32)
    mvar = np.asarray(inputs["moving_var"], dtype=np.float32)

    inv = (gamma.astype(np.float64) /
           np.sqrt(mvar.astype(np.float64) + BN_EPS)).astype(np.float32)
    cvec = (beta - mmean * inv).astype(np.float32)
    # BN folds into the ACT exactly when the affine addend is zero and the
    # scale is non-negative (Prelu commutes with positive scaling)
    fused = bool(np.all(cvec == 0.0) and np.all(inv >= 0.0))

    in_maps, metas = _prep_inputs(x, gidx, W, b, gamma, beta, mmean, mvar,
                                  fused)
    nc = _get_program(fused)

    from concourse import bass_utils
    res = bass_utils.run_bass_kernel_spmd(
        nc, in_maps, core_ids=list(range(N_CORES)), trace=TRACE, **TRACE_KW)
    if TRACE:
        kernel.last_result = res

    return _unshard(res.results, metas, b, gamma, beta, mmean, mvar)
